# revision 92
# baseline (speedup 1.0000x reference)
"""Trainium2 Bass kernel: causal MultiHeadAttention with RoPE (head-parallel).

B=1, S=4096, D=768, H=12 heads, dk=64, fp32 I/O. 8 NeuronCores, SPMD.

Sharding: head-parallel with split tail heads. Core c owns whole head
A=c (heads 0..7) plus HALF of head B=8+c//2: the q-tiles of parity c%2
(4 tiles of 256 rows each). Every core projects K/V/Q for its two heads
over the full sequence, runs full-causal attention for head A and its
four B q-tiles (identical instruction stream everywhere; the lone
parity-dependent bit is a [128,1024] mask input), computes bf16 partial
output projections, and combines them with ReduceScatter collectives:
an 8-way RS for the A-partials (q-sharded result) and a 4-way RS over
each parity group for the B-partials, which the host adds on top.
"""

import sys

if "/opt/trn_rl_repo" not in sys.path:
    sys.path.insert(0, "/opt/trn_rl_repo")

import numpy as np
import ml_dtypes

D_MODEL = 768
H = 12
DK = 64
S = 4096
THETA = 10000.0
MAX_SEQ_LEN = 4096
N_CORES = 8
EB = D_MODEL // 128   # 6 contraction blocks
N_CH = S // 512       # 8 sequence chunks
VW = 161              # V_aug s-tile layout: [Adata 64|Aone|Bone|fill 31|Bdata 64]
#  - A-PV lhsT window = cols 0:65   -> out rows 0:64 data, 64 denom
#  - B-PV lhsT window = cols 33:161 -> out row 32 denom, rows 64:128 data
#    (junk rows 0:32,33:64 unused; alignment puts B attn at partitions
#     64:128 so o_proj contracts A+B in ONE 128-deep matmul)

BF16 = ml_dtypes.bfloat16
FP8 = ml_dtypes.float8_e4m3   # matches mybir.dt.float8e4

# ReduceScatter chunks: fire after q-tile T_FIRE, covering q rows [lo, hi).
# Fired one q-tile after the covered rows complete so the collective's
# input-DMA waits are already satisfied and never block the Pool queue.
# Each RS writes its shard straight into the bf16 external output; chunk
# boundaries are chosen so each RS clears the (exclusive) collective
# device before the next one fires: cost = 15us flat + out_bytes/40GB/s.
RS_CHUNKS = [
    (11, 0, 2816),
    (13, 2816, 3584),
    (15, 3584, 4096),
]
# out_d row offset of each chunk's shard
RS_OUT_OFF = [0, 352, 448]


def build_program(with_rs=True):
    import concourse.mybir as mybir
    import concourse.tile as tile
    from concourse import bacc, library_config
    from concourse.tile import add_dep_helper

    f32 = mybir.dt.float32
    bf16 = mybir.dt.bfloat16
    fp8 = mybir.dt.float8e4
    DR = mybir.MatmulPerfMode.DoubleRow
    Exp = mybir.ActivationFunctionType.Exp
    Copy = mybir.ActivationFunctionType.Copy

    nc = bacc.Bacc(
        "TRN2",
        target_bir_lowering=False,
        debug=False,
        enable_asserts=True,
        num_devices=N_CORES,
    )

    xt_d = nc.dram_tensor("xt", [D_MODEL, S], bf16, kind="ExternalInput")
    xt8_d = nc.dram_tensor("xt8", [D_MODEL, S], fp8, kind="ExternalInput")
    w_d = {
        n: nc.dram_tensor(n, [D_MODEL, 128], bf16, kind="ExternalInput")
        for n in ("wv2",)
    }
    w8_d = {
        n: nc.dram_tensor(n, [D_MODEL, 128], fp8, kind="ExternalInput")
        for n in ("wq8", "wk8", "wq8s", "wk8s")
    }
    wo_d = nc.dram_tensor("wo2", [128, D_MODEL], bf16, kind="ExternalInput")
    onesp_d = nc.dram_tensor("onesp", [64, 128], bf16, kind="ExternalInput")
    idxb_d = nc.dram_tensor(
        "idxb", [128, N_CH], mybir.dt.int16, kind="ExternalInput"
    )
    maskh_d = nc.dram_tensor("maskh", [128, 1024], bf16, kind="ExternalInput")
    cosk_d = nc.dram_tensor("cosk", [128, S], bf16, kind="ExternalInput")
    sink_d = nc.dram_tensor("sink", [128, S], bf16, kind="ExternalInput")
    mask_d = nc.dram_tensor("maskab", [128, 512], bf16, kind="ExternalInput")
    out_d = nc.dram_tensor("out", [512, D_MODEL], bf16, kind="ExternalOutput")
    o_part = nc.dram_tensor("o_part", [S, D_MODEL], bf16, kind="Internal")
    ors = [
        nc.dram_tensor(f"ors{j}", [(hi - lo) // 8, D_MODEL], bf16, kind="Internal")
        for j, (_, lo, hi) in enumerate(RS_CHUNKS)
    ]


    PAIRSWAP = [i ^ 1 for i in range(32)]

    with tile.TileContext(nc) as tc:
        with (
            tc.tile_pool(name="const", bufs=1) as cpool,
            tc.tile_pool(name="rope", bufs=6) as rpool,
            tc.tile_pool(name="expp", bufs=8) as epool,
            tc.tile_pool(name="norm", bufs=8) as npool,
            tc.tile_pool(name="rsrb", bufs=2) as rbpool,
            tc.tile_pool(name="ps", bufs=3, space="PSUM") as pspool,
            tc.tile_pool(name="ps_pv", bufs=2, space="PSUM") as psv,
        ):
            nc.gpsimd.load_library(library_config.ap_gather)

            # ---- persistent tensors; first chunk's inputs loaded first ----
            def load_w(n):
                t = cpool.tile([128, EB * 128], bf16, tag=f"w_{n}", name=n)
                nc.sync.dma_start(
                    out=t[:].rearrange("p (e m) -> p e m", m=128),
                    in_=w_d[n][:].rearrange("(e p) m -> p e m", p=128),
                )
                return t

            def load_w8(n):
                t = cpool.tile([128, EB * 128], fp8, tag=f"w_{n}", name=n)
                nc.sync.dma_start(
                    out=t[:].rearrange("p (e m) -> p e m", m=128),
                    in_=w8_d[n][:].rearrange("(e p) m -> p e m", p=128),
                )
                return t

            def xt_load(lo, hi):
                nc.sync.dma_start(
                    out=xt_sb[:].rearrange("p (e s) -> p e s", s=S)[:, :, lo:hi],
                    in_=xt_d[:].rearrange("(e p) s -> p e s", p=128)[:, :, lo:hi],
                )

            def xt8_load(lo, hi):
                nc.sync.dma_start(
                    out=xt8_sb[:].rearrange("p (e s) -> p e s", s=S)[:, :, lo:hi],
                    in_=xt8_d[:].rearrange("(e p) s -> p e s", p=128)[:, :, lo:hi],
                )

            def cs_load(lo, hi):
                nc.sync.dma_start(out=cosk_sb[:, lo:hi], in_=cosk_d[:, lo:hi])
                nc.sync.dma_start(out=sink_sb[:, lo:hi], in_=sink_d[:, lo:hi])

            # chunk-0 K/Q inputs first, then V, then head-B / masks / wo,
            # then remaining chunks interleaved in first-use order
            wk_sb = load_w8("wk8")
            wks_sb = load_w8("wk8s")
            xt8_sb = cpool.tile([128, EB * S], fp8, tag="xt8_sb")
            xt8_load(0, 512)
            cosk_sb = cpool.tile([128, S], bf16, tag="cosk_sb")
            sink_sb = cpool.tile([128, S], bf16, tag="sink_sb")
            cs_load(0, 512)
            wq_sb = load_w8("wq8")
            wqs_sb = load_w8("wq8s")
            wv_sb = load_w("wv2")
            xt_sb = cpool.tile([128, EB * S], bf16, tag="xt_sb")
            xt_load(0, 512)
            maskab = cpool.tile([128, 512], bf16, tag="maskab")
            nc.sync.dma_start(out=maskab[:], in_=mask_d[:])
            idxb_sb = cpool.tile([128, N_CH], mybir.dt.int16, tag="idxb")
            nc.sync.dma_start(out=idxb_sb[:], in_=idxb_d[:])
            xt8_load(512, 1024)
            cs_load(512, 1024)
            maskh = cpool.tile([128, 1024], bf16, tag="maskh")
            nc.sync.dma_start(out=maskh[:], in_=maskh_d[:])
            xt_load(512, 1024)
            wo_sb = cpool.tile([128, D_MODEL], bf16, tag="wo2")
            nc.sync.dma_start(out=wo_sb[:], in_=wo_d[:])
            onesp_sb = cpool.tile([64, 128], bf16, tag="onesp")
            nc.sync.dma_start(out=onesp_sb[:], in_=onesp_d[:])
            for ch in range(2, N_CH):
                xt8_load(ch * 512, (ch + 1) * 512)
                cs_load(ch * 512, (ch + 1) * 512)
                xt_load(ch * 512, (ch + 1) * 512)

            qb = cpool.tile([128, 2048], bf16, tag="qb")
            ones64 = cpool.tile([65, 64], bf16, tag="ones64")
            nc.vector.memset(ones64[64:65, :], 1.0)
            qt = cpool.tile([128, S], bf16, tag="qt")
            ktc = [
                cpool.tile([128, 512], bf16, tag=f"kt{ch}", name=f"kt{ch}")
                for ch in range(N_CH)
            ]
            vc = [
                cpool.tile([128, 4 * VW], bf16, tag=f"va{ch}", name=f"va{ch}")
                for ch in range(N_CH)
            ]
            attnAB = cpool.tile([128, S], bf16, tag="attnAB", name="attnAB")

            def rope(dst, src_ps, swp_ps, cos_ap, sin_ap):
                # pair-swap comes from a second projection against
                # host-swapped weights; both muls read PSUM directly
                xb = rpool.tile([128, 512], bf16, tag="rope_x")
                nc.vector.tensor_mul(xb[:], src_ps, cos_ap)
                sh = rpool.tile([128, 512], bf16, tag="rope_sh")
                nc.vector.tensor_mul(sh[:], swp_ps, sin_ap)
                nc.vector.tensor_add(dst, xb[:], sh[:])

            # ---- warm-chained attention tiles: each tile pre-issues the
            # NEXT tile's first score-group + exp before its own final PV
            # batch, so the Act engine never drains at tile boundaries.
            def spec_a(T):
                return ("A", T,
                        [(pg, 2 if pg + 1 <= T else 1)
                         for pg in range(0, T + 1, 2)])

            def spec_b(m):
                return ("B", m, [(pg, 2) for pg in range(0, 2 * m + 2, 2)])

            def issue_group(spec, pg, w):
                """Score matmuls + exp (+ causal mask) for one group."""
                kind, idx, _ = spec
                ro = 0 if kind == "A" else 64
                if kind == "A":
                    qsl = qt[0:64, idx * 256:idx * 256 + 256]
                else:
                    qsl = qb[64:128, idx * 256:idx * 256 + 256]
                sc = pspool.tile([128, 1024], f32, tag="ps")
                for pi in range(w):
                    for j in range(2):
                        t = 2 * (pg + pi) + j
                        nc.tensor.matmul(
                            sc[:, (2 * pi + j) * 256:(2 * pi + j + 1) * 256],
                            ktc[t // 4][ro:ro + 64, (t % 4) * 128:(t % 4) * 128 + 128],
                            qsl,
                            start=True,
                            stop=True,
                        )
                et = epool.tile([128, 1024], bf16, tag="et")
                nc.scalar.activation(
                    et[:, 0:512 * w], sc[:, 0:512 * w], Exp, bias=0.0, scale=0.125
                )
                if kind == "A":
                    if pg + w - 1 == idx:  # group holds the diagonal pair
                        off = 512 * (w - 1)
                        nc.vector.tensor_mul(
                            et[:, off:off + 512], et[:, off:off + 512], maskab[:]
                        )
                else:
                    if pg + 2 >= 2 * idx + 2:  # diagonal + padding pair
                        nc.vector.tensor_mul(et[:], et[:], maskh[:])
                return et

            def issue_pv(spec, pv, pg, w, et):
                kind, idx, _ = spec
                last_p = idx if kind == "A" else 2 * idx + 1
                for pi in range(w):
                    p = pg + pi
                    for j in range(2):
                        t = 2 * p + j
                        if kind == "A":
                            lhsT = vc[t // 4][:, (t % 4) * VW:(t % 4) * VW + 65]
                        else:
                            lhsT = vc[t // 4][:, (t % 4) * VW + 33:(t % 4) * VW + 161]
                        nc.tensor.matmul(
                            pv[:],
                            lhsT,
                            et[:, (2 * pi + j) * 256:(2 * pi + j + 1) * 256],
                            start=(p == 0 and j == 0),
                            stop=(p == last_p and j == 1),
                        )

            def run_tile(spec, warm, next_spec):
                kind, idx, groups = spec
                pv = psv.tile(
                    [65 if kind == "A" else 128, 256], f32, tag="ps_pv"
                )
                # up to 2 groups were pre-issued by the previous tile;
                # PV issue lags the exp issue by the same depth
                pend = []
                if warm:
                    for (pg, w), et in zip(groups, warm):
                        pend.append((pg, w, et))
                for pg, w in groups[len(pend):]:
                    et = issue_group(spec, pg, w)
                    if pend:
                        issue_pv(spec, pv, *pend.pop(0))
                    pend.append((pg, w, et))
                warm_next = None
                if next_spec is not None:
                    # small (early) tiles are pre-issued in full to keep
                    # the Act engine fed through the DMA-bound startup;
                    # big tiles cap at 3 groups to limit PSUM pressure
                    nd = 4 if len(next_spec[2]) <= 4 else 3
                    warm_next = [
                        issue_group(next_spec, pg, w)
                        for pg, w in next_spec[2][:nd]
                    ]
                for h in pend:
                    issue_pv(spec, pv, *h)
                # reduce + reciprocal now; broadcast and normalize deferred
                if kind == "A":
                    pvs = npool.tile([65, 256], f32, tag="pvs")
                    nc.vector.tensor_copy(pvs[:], pv[:])
                    rrow = npool.tile([65, 256], bf16, tag="rrow")
                    with nc.allow_low_precision(reason="bf16 softmax denom"):
                        nc.vector.reciprocal(rrow[64:65, :], pvs[64:65, :])
                else:
                    pvs = npool.tile([128, 256], f32, tag="pvs")
                    nc.vector.tensor_copy(pvs[64:128, :], pv[64:128, :])
                    rrow = npool.tile([65, 256], bf16, tag="rrow")
                    with nc.allow_low_precision(reason="bf16 softmax denom"):
                        # denominator read straight from PSUM row 32
                        nc.vector.reciprocal(rrow[32:33, :], pv[32:33, :])
                return (pvs, rrow), warm_next

            def normalize(pvs, rrow, dst):
                rb = psv.tile([65, 256], f32, tag="ps_pv")
                nc.tensor.matmul(
                    rb[0:64, :], ones64[64:65, :], rrow[64:65, :],
                    start=True, stop=True,
                )
                nc.vector.tensor_mul(dst, pvs[0:64, :], rb[0:64, :])

            def finalize_tile(T, handles):
                """Deferred normalize (broadcast via K=1 matmul) + output
                projection for q-tile T; issued one tile later so the
                reciprocal is ready and the PE never waits.  The last tile
                pipelines normalize/o_proj/copy/DMA per 128-row half so the
                final ReduceScatter fires as early as possible."""
                pvs, rrow = handles
                if T < N_CH * 2 - 1:
                    normalize(pvs, rrow, attnAB[0:64, T * 256:T * 256 + 256])
                    o_proj_pair(T)
                    return
                rb = psv.tile([65, 256], f32, tag="ps_pv")
                nc.tensor.matmul(
                    rb[0:64, :], ones64[64:65, :], rrow[64:65, :],
                    start=True, stop=True,
                )
                osb = rbpool.tile([128, 2 * D_MODEL], bf16, tag="osb")
                for half in range(2):
                    hs = slice(half * 128, (half + 1) * 128)
                    nc.vector.tensor_mul(
                        attnAB[0:64, T * 256 + half * 128:T * 256 + half * 128 + 128],
                        pvs[0:64, hs], rb[0:64, hs],
                    )
                    qtl = 2 * T + half
                    pot = pspool.tile([128, 1024], f32, tag="ps")
                    lhsAB = attnAB[:, qtl * 128:qtl * 128 + 128]
                    nc.tensor.matmul(
                        pot[:, 0:512], lhsAB, wo_sb[:, 0:512],
                        start=True, stop=True,
                    )
                    nc.tensor.matmul(
                        pot[:, 512:768], lhsAB, wo_sb[:, 512:768],
                        start=True, stop=True,
                    )
                    if half == 0:
                        nc.scalar.activation(
                            osb[:, 0:768], pot[:, 0:768],
                            Copy, bias=0.0, scale=1.0,
                        )
                    else:
                        nc.vector.tensor_copy(osb[:, 768:1536], pot[:, 0:768])
                    r0 = T * 256 + half * 128
                    last_opart[0] = nc.sync.dma_start(
                        out=o_part[r0:r0 + 128, :],
                        in_=osb[:, half * D_MODEL:(half + 1) * D_MODEL],
                    )

            def finalize_btile(m, handles):
                """Write the B attention into attnAB rows 64:128 at BOTH
                physical positions 2m and 2m+1; the parity one-hot in the
                broadcast ones-column (host data) zeroes the non-owned
                position so o_proj can use real head-B weights everywhere."""
                pvs, rrow = handles
                for po in range(2):
                    pos = 2 * m + po
                    rb = psv.tile([128, 256], f32, tag="ps_pv")
                    nc.tensor.matmul(
                        rb[64:128, :],
                        onesp_sb[32:33, po * 64:(po + 1) * 64],
                        rrow[32:33, :],
                        start=True, stop=True,
                    )
                    nc.vector.tensor_mul(
                        attnAB[64:128, pos * 256:(pos + 1) * 256],
                        pvs[64:128, :], rb[64:128, :],
                    )

            def o_proj_pair(T):
                osb = rbpool.tile([128, 2 * D_MODEL], bf16, tag="osb")
                for half in range(2):
                    qtl = 2 * T + half
                    pot = pspool.tile([128, 1024], f32, tag="ps")
                    # A (rows 0:64) and B (rows 64:128) contract together
                    lhsAB = attnAB[:, qtl * 128:qtl * 128 + 128]
                    nc.tensor.matmul(
                        pot[:, 0:512], lhsAB, wo_sb[:, 0:512],
                        start=True, stop=True,
                    )
                    nc.tensor.matmul(
                        pot[:, 512:768], lhsAB, wo_sb[:, 512:768],
                        start=True, stop=True,
                    )
                    # Act engine has slack during the small early tiles and
                    # DVE is the early bottleneck; late tiles are the
                    # reverse.  The very last tile splits across both
                    # engines so the final ReduceScatter fires sooner.
                    if (T < 10 or T >= 14) and not (
                        T == N_CH * 2 - 1 and half == 1
                    ):
                        nc.scalar.activation(
                            osb[:, half * D_MODEL:(half + 1) * D_MODEL],
                            pot[:, 0:768], Copy, bias=0.0, scale=1.0,
                        )
                    else:
                        nc.vector.tensor_copy(
                            osb[:, half * D_MODEL:(half + 1) * D_MODEL],
                            pot[:, 0:768],
                        )
                    # per-half DMA so the last half's write (and the final
                    # ReduceScatter behind it) starts as early as possible
                    r0 = T * 256 + half * 128
                    last_opart[0] = nc.sync.dma_start(
                        out=o_part[r0:r0 + 128, :],
                        in_=osb[:, half * D_MODEL:(half + 1) * D_MODEL],
                    )

            def fire_rs(j):
                _, lo, hi = RS_CHUNKS[j]
                nc.gpsimd.collective_compute(
                    "ReduceScatter",
                    mybir.AluOpType.add,
                    replica_groups=[list(range(N_CORES))],
                    ins=[o_part[lo:hi, :]],
                    outs=[ors[j][:]],
                )

            def readback(j):
                """Pure-DMA bounce ors -> SBUF -> out (both bf16)."""
                _, lo, hi = RS_CHUNKS[j]
                shard = (hi - lo) // 8
                oo = RS_OUT_OFF[j]
                rt = rbpool.tile([128, 4 * D_MODEL], bf16, tag="rt")
                rd = nc.sync.dma_start(
                    out=rt[0:shard // 4, :].rearrange(
                        "p (h d) -> p h d", d=D_MODEL
                    ),
                    in_=ors[j][:].rearrange("(p h) d -> p h d", h=4),
                )
                # keep readback DMAs after all o_part writes so the
                # round-robin DMA-queue counts of collective waits never
                # include collective-dependent transfers
                if last_opart[0] is not None:
                    add_dep_helper(
                        rd.ins, last_opart[0].ins, sync=True,
                        reason="readback after o_part stream",
                    )
                nc.sync.dma_start(
                    out=out_d[oo:oo + shard, :].rearrange(
                        "(p h) d -> p h d", h=4
                    ),
                    in_=rt[0:shard // 4, :].rearrange(
                        "p (h d) -> p h d", d=D_MODEL
                    ),
                )

            # ---- main loop: projection chunk ch, then attention q-tiles;
            # tile finalization (normalize + o_proj + RS) runs one tile behind
            rs_next = 0
            last_opart = [None]
            b_done = False
            pending = None  # (kind, idx, handles)

            def proj_pair(ch, w, ws, dst):
                # fp8 DoubleRow: each matmul contracts a PAIR of 128-row
                # feature blocks (weights/ifmap free dims are (2, m)/(2, s));
                # a tensor and its pair-swapped twin share one PSUM tile,
                # consumed by rope before the next pair allocates
                xt8_3d = xt8_sb[:].rearrange("p (e s) -> p e s", s=S)
                ck = cosk_sb[:, ch * 512:(ch + 1) * 512]
                sk = sink_sb[:, ch * 512:(ch + 1) * 512]
                ps = pspool.tile([128, 1024], f32, tag="ps")
                for off, wt in ((0, w), (512, ws)):
                    w3 = wt[:].rearrange("p (e m) -> p e m", m=128)
                    for j in range(EB // 2):
                        nc.tensor.matmul(
                            ps[:, off:off + 512],
                            w3[:, 2 * j:2 * j + 2, :],
                            xt8_3d[:, 2 * j:2 * j + 2, ch * 512:(ch + 1) * 512],
                            start=(j == 0),
                            stop=(j == EB // 2 - 1),
                            perf_mode=DR,
                        )
                rope(dst, ps[:, 0:512], ps[:, 512:1024], ck, sk)

            def do_proj_kq(ch):
                proj_pair(ch, wk_sb, wks_sb, ktc[ch][:])
                proj_pair(ch, wq_sb, wqs_sb, qt[:, ch * 512:(ch + 1) * 512])

            def do_proj_v(ch):
                def xt_t_slice(eb, lo, hi):
                    return xt_sb[:, eb * S + ch * 512 + lo:eb * S + ch * 512 + hi]
                # V (natural layout, interleaved ones column per head):
                # 4 s-tiles accumulate into one PSUM tile
                psV4 = pspool.tile([128, 1024], f32, tag="ps")
                for stl in range(4):
                    for eb in range(EB):
                        nc.tensor.matmul(
                            psV4[:, stl * 256:stl * 256 + 128],
                            xt_t_slice(eb, stl * 128, stl * 128 + 128),
                            wv_sb[:, eb * 128:(eb + 1) * 128],
                            start=(eb == 0),
                            stop=(eb == EB - 1),
                        )
                vall = vc[ch][:].rearrange("p (s x) -> p s x", s=4)
                nc.vector.memset(vall[:, :, 64:66], 1.0)   # Aone, Bone
                nc.vector.memset(vall[:, :, 66:97], 0.0)   # fill
                vsrc = psV4[:].rearrange("p (s x) -> p s x", s=4)[:, :, 0:128]
                # A data at cols 0:64, B data at cols 97:161
                nc.vector.tensor_copy(vall[:, :, 0:64], vsrc[:, :, 0:64])
                nc.vector.tensor_copy(vall[:, :, 97:161], vsrc[:, :, 64:128])

            def gather_qb(m):
                """qb tile m = roped head-B Q of physical q-tile 2m+parity,
                gathered from qt's 512-col window holding both parities'
                tiles (indices are per-core host data, so the instruction
                stream stays SPMD-uniform and m-independent)."""
                nc.gpsimd.ap_gather(
                    out_ap=qb[:, m * 256:(m + 1) * 256].rearrange(
                        "p (i d) -> p i d", d=16
                    ),
                    in_ap=qt[:, m * 512:(m + 1) * 512].rearrange(
                        "p (i d) -> p i d", d=16
                    ),
                    idxs_ap=idxb_sb[:, 0:1],
                    channels=128,
                    num_elems=32,
                    d=16,
                    num_idxs=16,
                )

            # projections run one chunk ahead, split and issued mid-chunk so
            # the attention tiles keep the Act engine fed at boundaries
            do_proj_kq(0)
            do_proj_v(0)
            gather_qb(0)
            do_proj_kq(1)
            do_proj_v(1)
            gather_qb(1)

            def pop_pending():
                nonlocal pending, rs_next, b_done
                if pending is None:
                    return
                kind, idx, ph = pending
                if kind == "A":
                    finalize_tile(idx, ph)
                    if (
                        with_rs
                        and rs_next < len(RS_CHUNKS)
                        and RS_CHUNKS[rs_next][0] == idx
                    ):
                        fire_rs(rs_next)
                        rs_next += 1
                else:
                    finalize_btile(idx, ph)
                pending = None

            warm = None
            for ch in range(N_CH):
                bh, warm = run_tile(spec_b(ch), warm, spec_a(2 * ch))
                if ch + 2 < N_CH:
                    do_proj_v(ch + 2)
                pop_pending()
                pending = ("B", ch, bh)
                for T in (2 * ch, 2 * ch + 1):
                    if T == 2 * ch:
                        nxt = spec_a(T + 1)
                    elif ch + 1 < N_CH:
                        nxt = spec_b(ch + 1)
                    else:
                        nxt = None
                    handles, warm = run_tile(spec_a(T), warm, nxt)
                    pop_pending()
                    pending = ("A", T, handles)
                    if T == 13:
                        # finalize now so the middle ReduceScatter fires
                        # before b7's issue stream and clears the exclusive
                        # collective device ahead of the final chunk
                        pop_pending()
                    if ch + 2 < N_CH:
                        if T == 2 * ch:
                            proj_pair(
                                ch + 2, wk_sb, wks_sb, ktc[ch + 2][:]
                            )
                        else:
                            proj_pair(
                                ch + 2, wq_sb, wqs_sb,
                                qt[:, (ch + 2) * 512:(ch + 3) * 512],
                            )
                            gather_qb(ch + 2)
            pop_pending()
            while with_rs and rs_next < len(RS_CHUNKS):
                fire_rs(rs_next)
                rs_next += 1
            if with_rs:
                for j in range(len(RS_CHUNKS)):
                    readback(j)


    nc.compile()
    return nc


_PROGRAM = None


def _get_program():
    global _PROGRAM
    if _PROGRAM is None:
        _PROGRAM = build_program()
    return _PROGRAM


def host_prep(in_features, token_positions, q_proj, k_proj, v_proj, o_proj):
    """Build the 8 per-core input maps."""
    x = np.asarray(in_features, np.float32).reshape(S, D_MODEL)
    tp = np.asarray(token_positions)
    qp = np.asarray(q_proj, np.float32)
    kp = np.asarray(k_proj, np.float32)
    vp = np.asarray(v_proj, np.float32)
    op = np.asarray(o_proj, np.float32)

    xt_bf = np.ascontiguousarray(x.T).astype(BF16)      # [768, 4096]
    xt_f8 = np.ascontiguousarray(x.T).astype(FP8)
    wqT = np.ascontiguousarray(qp.T)                    # [in 768, out 768]
    wkT = np.ascontiguousarray(kp.T)
    wvT = np.ascontiguousarray(vp.T)
    opT = np.ascontiguousarray(op.T)                    # [in-dk 768, out 768]

    inv_freq = 1.0 / THETA ** (np.arange(0, DK, 2, dtype=np.float32) / DK)
    pos = np.clip(tp.astype(np.float32), 0, MAX_SEQ_LEN - 1)
    freq = pos[:, None] * inv_freq[None, :]             # [S, 32]
    cos_t, sin_t = np.cos(freq), np.sin(freq)

    r = np.arange(128)
    fidx = (r % 64) // 2
    sign = np.where(r % 2 == 0, -1.0, 1.0).astype(np.float32)
    cos128 = cos_t[:, fidx].T.astype(BF16)              # [128, S]
    sin128 = (sin_t[:, fidx].T * sign[:, None]).astype(BF16)

    ki = np.arange(128)[:, None]
    qi = np.arange(256)[None, :]
    mask_a = (ki <= qi).astype(np.float32)
    mask_b = (ki + 128 <= qi).astype(np.float32)
    maskab = np.concatenate([mask_a, mask_b], axis=1).astype(BF16)

    ones512 = np.ones((128, 512), np.float32)
    zeros512 = np.zeros((128, 512), np.float32)

    in_maps = []
    for c in range(N_CORES):
        hA = c
        hB = 8 + c // 2
        p = c % 2

        def wslice(wT, dt=BF16):
            out = np.empty((D_MODEL, 128), np.float32)
            out[:, 0:64] = wT[:, hA * 64:(hA + 1) * 64]
            out[:, 64:128] = wT[:, hB * 64:(hB + 1) * 64]
            return out.astype(dt)

        wo2 = np.zeros((128, D_MODEL), np.float32)
        wo2[0:64, :] = opT[hA * 64:(hA + 1) * 64, :]
        wo2[64:128, :] = opT[hB * 64:(hB + 1) * 64, :]

        # parity one-hot ones-columns used to broadcast (and zero) the
        # head-B softmax reciprocals at positions 2m / 2m+1
        onesp = np.zeros((64, 128), np.float32)
        onesp[32, 0:64] = 1.0 if p == 0 else 0.0
        onesp[32, 64:128] = 1.0 if p == 1 else 0.0

        # ap_gather block indices: within each 512-col qt window, this
        # core's parity selects the first or second 256-col q-tile
        # (rows 0:64 gather head-A data into unused qb rows; keep valid)
        idxb = np.zeros((128, N_CH), np.int16)
        idxb[0:64, :] = (np.arange(64) % 16)[:, None]
        idxb[64:128, :] = (p * 16 + np.arange(64) % 16)[:, None]

        maskab_f = maskab.astype(np.float32)
        if p == 0:
            maskh = np.concatenate([maskab_f, zeros512], axis=1)
        else:
            maskh = np.concatenate([ones512, maskab_f], axis=1)

        in_maps.append(
            {
                "xt": xt_bf,
                "xt8": xt_f8,
                "idxb": idxb,
                "wq8": wslice(wqT, FP8),
                "wk8": wslice(wkT, FP8),
                "wq8s": wslice(wqT, FP8)[:, np.arange(128) ^ 1],
                "wk8s": wslice(wkT, FP8)[:, np.arange(128) ^ 1],
                "wv2": wslice(wvT),
                "wo2": wo2.astype(BF16),
                "onesp": onesp.astype(BF16),
                "cosk": cos128,
                "sink": sin128,
                "maskab": maskab,
                "maskh": maskh.astype(BF16),
            }
        )
    return in_maps


def assemble_output(results):
    out = np.empty((1, S, D_MODEL), np.float32)
    for c in range(N_CORES):
        r = np.asarray(results[c]["out"]).astype(np.float32)
        for j, (_, lo, hi) in enumerate(RS_CHUNKS):
            shard = (hi - lo) // 8
            oo = RS_OUT_OFF[j]
            out[0, lo + shard * c:lo + shard * (c + 1)] = r[oo:oo + shard]
    return out


def kernel(**inputs):
    from concourse.bass_utils import run_bass_kernel_spmd

    nc = _get_program()
    in_maps = host_prep(**inputs)
    res = run_bass_kernel_spmd(nc, in_maps, list(range(N_CORES)))
    return assemble_output(res.results)


if __name__ == "__main__":
    nc = build_program()
    print("program built and compiled")



# revision 96
# speedup vs baseline: 1.0008x; 1.0008x over previous
"""Trainium2 Bass kernel: causal MultiHeadAttention with RoPE (head-parallel).

B=1, S=4096, D=768, H=12 heads, dk=64, fp32 I/O. 8 NeuronCores, SPMD.

Sharding: head-parallel with split tail heads. Core c owns whole head
A=c (heads 0..7) plus HALF of head B=8+c//2: the q-tiles of parity c%2
(4 tiles of 256 rows each). Every core projects K/V/Q for its two heads
over the full sequence, runs full-causal attention for head A and its
four B q-tiles (identical instruction stream everywhere; the lone
parity-dependent bit is a [128,1024] mask input), computes bf16 partial
output projections, and combines them with ReduceScatter collectives:
an 8-way RS for the A-partials (q-sharded result) and a 4-way RS over
each parity group for the B-partials, which the host adds on top.
"""

import sys

if "/opt/trn_rl_repo" not in sys.path:
    sys.path.insert(0, "/opt/trn_rl_repo")

import numpy as np
import ml_dtypes

D_MODEL = 768
H = 12
DK = 64
S = 4096
THETA = 10000.0
MAX_SEQ_LEN = 4096
N_CORES = 8
EB = D_MODEL // 128   # 6 contraction blocks
N_CH = S // 512       # 8 sequence chunks
VW = 161              # V_aug s-tile layout: [Adata 64|Aone|Bone|fill 31|Bdata 64]
#  - A-PV lhsT window = cols 0:65   -> out rows 0:64 data, 64 denom
#  - B-PV lhsT window = cols 33:161 -> out row 32 denom, rows 64:128 data
#    (junk rows 0:32,33:64 unused; alignment puts B attn at partitions
#     64:128 so o_proj contracts A+B in ONE 128-deep matmul)

BF16 = ml_dtypes.bfloat16
FP8 = ml_dtypes.float8_e4m3   # matches mybir.dt.float8e4

# ReduceScatter chunks: fire after q-tile T_FIRE, covering q rows [lo, hi).
# Fired one q-tile after the covered rows complete so the collective's
# input-DMA waits are already satisfied and never block the Pool queue.
# Each RS writes its shard straight into the bf16 external output; chunk
# boundaries are chosen so each RS clears the (exclusive) collective
# device before the next one fires: cost = 15us flat + out_bytes/40GB/s.
RS_CHUNKS = [
    (11, 0, 2816),
    (13, 2816, 3584),
    (15, 3584, 4096),
]
# out_d row offset of each chunk's shard
RS_OUT_OFF = [0, 352, 448]


def build_program(with_rs=True):
    import concourse.mybir as mybir
    import concourse.tile as tile
    from concourse import bacc, library_config
    from concourse.tile import add_dep_helper

    f32 = mybir.dt.float32
    bf16 = mybir.dt.bfloat16
    fp8 = mybir.dt.float8e4
    DR = mybir.MatmulPerfMode.DoubleRow
    Exp = mybir.ActivationFunctionType.Exp
    Copy = mybir.ActivationFunctionType.Copy

    nc = bacc.Bacc(
        "TRN2",
        target_bir_lowering=False,
        debug=False,
        enable_asserts=True,
        num_devices=N_CORES,
    )

    xt_d = nc.dram_tensor("xt", [D_MODEL, S], bf16, kind="ExternalInput")
    xt8_d = nc.dram_tensor("xt8", [D_MODEL, S], fp8, kind="ExternalInput")
    w_d = {
        n: nc.dram_tensor(n, [D_MODEL, 128], bf16, kind="ExternalInput")
        for n in ("wv2",)
    }
    w8_d = {
        n: nc.dram_tensor(n, [D_MODEL, 128], fp8, kind="ExternalInput")
        for n in ("wq8", "wk8", "wq8s", "wk8s")
    }
    wo_d = nc.dram_tensor("wo2", [128, D_MODEL], bf16, kind="ExternalInput")
    onesp_d = nc.dram_tensor("onesp", [64, 128], bf16, kind="ExternalInput")
    idxb_d = nc.dram_tensor(
        "idxb", [128, N_CH], mybir.dt.int16, kind="ExternalInput"
    )
    maskh_d = nc.dram_tensor("maskh", [128, 1024], bf16, kind="ExternalInput")
    cosk_d = nc.dram_tensor("cosk", [128, S], bf16, kind="ExternalInput")
    sink_d = nc.dram_tensor("sink", [128, S], bf16, kind="ExternalInput")
    mask_d = nc.dram_tensor("maskab", [128, 512], bf16, kind="ExternalInput")
    out_d = nc.dram_tensor("out", [512, D_MODEL], bf16, kind="ExternalOutput")
    o_part = nc.dram_tensor("o_part", [S, D_MODEL], bf16, kind="Internal")
    ors = [
        nc.dram_tensor(f"ors{j}", [(hi - lo) // 8, D_MODEL], bf16, kind="Internal")
        for j, (_, lo, hi) in enumerate(RS_CHUNKS)
    ]


    PAIRSWAP = [i ^ 1 for i in range(32)]

    with tile.TileContext(nc) as tc:
        with (
            tc.tile_pool(name="const", bufs=1) as cpool,
            tc.tile_pool(name="rope", bufs=6) as rpool,
            tc.tile_pool(name="expp", bufs=10) as epool,
            tc.tile_pool(name="norm", bufs=8) as npool,
            tc.tile_pool(name="rsrb", bufs=2) as rbpool,
            tc.tile_pool(name="ps", bufs=3, space="PSUM") as pspool,
            tc.tile_pool(name="ps_pv", bufs=2, space="PSUM") as psv,
        ):
            nc.gpsimd.load_library(library_config.ap_gather)

            # ---- persistent tensors; first chunk's inputs loaded first ----
            def load_w(n):
                t = cpool.tile([128, EB * 128], bf16, tag=f"w_{n}", name=n)
                nc.sync.dma_start(
                    out=t[:].rearrange("p (e m) -> p e m", m=128),
                    in_=w_d[n][:].rearrange("(e p) m -> p e m", p=128),
                )
                return t

            def load_w8(n):
                t = cpool.tile([128, EB * 128], fp8, tag=f"w_{n}", name=n)
                nc.sync.dma_start(
                    out=t[:].rearrange("p (e m) -> p e m", m=128),
                    in_=w8_d[n][:].rearrange("(e p) m -> p e m", p=128),
                )
                return t

            def xt_load(lo, hi):
                nc.sync.dma_start(
                    out=xt_sb[:].rearrange("p (e s) -> p e s", s=S)[:, :, lo:hi],
                    in_=xt_d[:].rearrange("(e p) s -> p e s", p=128)[:, :, lo:hi],
                )

            def xt8_load(lo, hi):
                nc.sync.dma_start(
                    out=xt8_sb[:].rearrange("p (e s) -> p e s", s=S)[:, :, lo:hi],
                    in_=xt8_d[:].rearrange("(e p) s -> p e s", p=128)[:, :, lo:hi],
                )

            def cs_load(lo, hi):
                nc.sync.dma_start(out=cosk_sb[:, lo:hi], in_=cosk_d[:, lo:hi])
                nc.sync.dma_start(out=sink_sb[:, lo:hi], in_=sink_d[:, lo:hi])

            # chunk-0 K/Q inputs first, then V, then head-B / masks / wo,
            # then remaining chunks interleaved in first-use order
            wk_sb = load_w8("wk8")
            wks_sb = load_w8("wk8s")
            xt8_sb = cpool.tile([128, EB * S], fp8, tag="xt8_sb")
            xt8_load(0, 512)
            cosk_sb = cpool.tile([128, S], bf16, tag="cosk_sb")
            sink_sb = cpool.tile([128, S], bf16, tag="sink_sb")
            cs_load(0, 512)
            wq_sb = load_w8("wq8")
            wqs_sb = load_w8("wq8s")
            wv_sb = load_w("wv2")
            xt_sb = cpool.tile([128, EB * S], bf16, tag="xt_sb")
            xt_load(0, 512)
            maskab = cpool.tile([128, 512], bf16, tag="maskab")
            nc.sync.dma_start(out=maskab[:], in_=mask_d[:])
            idxb_sb = cpool.tile([128, N_CH], mybir.dt.int16, tag="idxb")
            nc.sync.dma_start(out=idxb_sb[:], in_=idxb_d[:])
            xt8_load(512, 1024)
            cs_load(512, 1024)
            maskh = cpool.tile([128, 1024], bf16, tag="maskh")
            nc.sync.dma_start(out=maskh[:], in_=maskh_d[:])
            xt_load(512, 1024)
            wo_sb = cpool.tile([128, D_MODEL], bf16, tag="wo2")
            nc.sync.dma_start(out=wo_sb[:], in_=wo_d[:])
            onesp_sb = cpool.tile([64, 128], bf16, tag="onesp")
            nc.sync.dma_start(out=onesp_sb[:], in_=onesp_d[:])
            for ch in range(2, N_CH):
                xt8_load(ch * 512, (ch + 1) * 512)
                cs_load(ch * 512, (ch + 1) * 512)
                xt_load(ch * 512, (ch + 1) * 512)

            qb = cpool.tile([128, 2048], bf16, tag="qb")
            ones64 = cpool.tile([65, 64], bf16, tag="ones64")
            nc.vector.memset(ones64[64:65, :], 1.0)
            qt = cpool.tile([128, S], bf16, tag="qt")
            ktc = [
                cpool.tile([128, 512], bf16, tag=f"kt{ch}", name=f"kt{ch}")
                for ch in range(N_CH)
            ]
            vc = [
                cpool.tile([128, 4 * VW], bf16, tag=f"va{ch}", name=f"va{ch}")
                for ch in range(N_CH)
            ]
            attnAB = cpool.tile([128, S], bf16, tag="attnAB", name="attnAB")

            def rope(dst, src_ps, swp_ps, cos_ap, sin_ap):
                # pair-swap comes from a second projection against
                # host-swapped weights; both muls read PSUM directly
                xb = rpool.tile([128, 512], bf16, tag="rope_x")
                nc.vector.tensor_mul(xb[:], src_ps, cos_ap)
                sh = rpool.tile([128, 512], bf16, tag="rope_sh")
                nc.vector.tensor_mul(sh[:], swp_ps, sin_ap)
                nc.vector.tensor_add(dst, xb[:], sh[:])

            # ---- warm-chained attention tiles: each tile pre-issues the
            # NEXT tile's first score-group + exp before its own final PV
            # batch, so the Act engine never drains at tile boundaries.
            def spec_a(T):
                return ("A", T,
                        [(pg, 2 if pg + 1 <= T else 1)
                         for pg in range(0, T + 1, 2)])

            def spec_b(m):
                return ("B", m, [(pg, 2) for pg in range(0, 2 * m + 2, 2)])

            def issue_group(spec, pg, w):
                """Score matmuls + exp (+ causal mask) for one group."""
                kind, idx, _ = spec
                ro = 0 if kind == "A" else 64
                if kind == "A":
                    qsl = qt[0:64, idx * 256:idx * 256 + 256]
                else:
                    qsl = qb[64:128, idx * 256:idx * 256 + 256]
                sc = pspool.tile([128, 1024], f32, tag="ps")
                for pi in range(w):
                    for j in range(2):
                        t = 2 * (pg + pi) + j
                        nc.tensor.matmul(
                            sc[:, (2 * pi + j) * 256:(2 * pi + j + 1) * 256],
                            ktc[t // 4][ro:ro + 64, (t % 4) * 128:(t % 4) * 128 + 128],
                            qsl,
                            start=True,
                            stop=True,
                        )
                et = epool.tile([128, 1024], bf16, tag="et")
                nc.scalar.activation(
                    et[:, 0:512 * w], sc[:, 0:512 * w], Exp, bias=0.0, scale=0.125
                )
                if kind == "A":
                    if pg + w - 1 == idx:  # group holds the diagonal pair
                        off = 512 * (w - 1)
                        nc.vector.tensor_mul(
                            et[:, off:off + 512], et[:, off:off + 512], maskab[:]
                        )
                else:
                    if pg + 2 >= 2 * idx + 2:  # diagonal + padding pair
                        nc.vector.tensor_mul(et[:], et[:], maskh[:])
                return et

            def issue_pv(spec, pv, pg, w, et):
                kind, idx, _ = spec
                last_p = idx if kind == "A" else 2 * idx + 1
                for pi in range(w):
                    p = pg + pi
                    for j in range(2):
                        t = 2 * p + j
                        if kind == "A":
                            lhsT = vc[t // 4][:, (t % 4) * VW:(t % 4) * VW + 65]
                        else:
                            lhsT = vc[t // 4][:, (t % 4) * VW + 33:(t % 4) * VW + 161]
                        nc.tensor.matmul(
                            pv[:],
                            lhsT,
                            et[:, (2 * pi + j) * 256:(2 * pi + j + 1) * 256],
                            start=(p == 0 and j == 0),
                            stop=(p == last_p and j == 1),
                        )

            def run_tile(spec, warm, next_spec):
                kind, idx, groups = spec
                pv = psv.tile(
                    [65 if kind == "A" else 128, 256], f32, tag="ps_pv"
                )
                # up to 2 groups were pre-issued by the previous tile;
                # PV issue lags the exp issue by the same depth
                pend = []
                if warm:
                    for (pg, w), et in zip(groups, warm):
                        pend.append((pg, w, et))
                for pg, w in groups[len(pend):]:
                    et = issue_group(spec, pg, w)
                    if pend:
                        issue_pv(spec, pv, *pend.pop(0))
                    pend.append((pg, w, et))
                warm_next = None
                if next_spec is not None:
                    warm_next = [
                        issue_group(next_spec, pg, w)
                        for pg, w in next_spec[2][:3]
                    ]
                for h in pend:
                    issue_pv(spec, pv, *h)
                # reduce + reciprocal now; broadcast and normalize deferred
                if kind == "A":
                    pvs = npool.tile([65, 256], f32, tag="pvs")
                    nc.vector.tensor_copy(pvs[:], pv[:])
                    rrow = npool.tile([65, 256], bf16, tag="rrow")
                    with nc.allow_low_precision(reason="bf16 softmax denom"):
                        nc.vector.reciprocal(rrow[64:65, :], pvs[64:65, :])
                else:
                    pvs = npool.tile([128, 256], f32, tag="pvs")
                    nc.vector.tensor_copy(pvs[64:128, :], pv[64:128, :])
                    rrow = npool.tile([65, 256], bf16, tag="rrow")
                    with nc.allow_low_precision(reason="bf16 softmax denom"):
                        # denominator read straight from PSUM row 32
                        nc.vector.reciprocal(rrow[32:33, :], pv[32:33, :])
                return (pvs, rrow), warm_next

            def normalize(pvs, rrow, dst):
                rb = psv.tile([65, 256], f32, tag="ps_pv")
                nc.tensor.matmul(
                    rb[0:64, :], ones64[64:65, :], rrow[64:65, :],
                    start=True, stop=True,
                )
                nc.vector.tensor_mul(dst, pvs[0:64, :], rb[0:64, :])

            def finalize_tile(T, handles):
                """Deferred normalize (broadcast via K=1 matmul) + output
                projection for q-tile T; issued one tile later so the
                reciprocal is ready and the PE never waits.  The last tile
                pipelines normalize/o_proj/copy/DMA per 128-row half so the
                final ReduceScatter fires as early as possible."""
                pvs, rrow = handles
                if T < N_CH * 2 - 1:
                    normalize(pvs, rrow, attnAB[0:64, T * 256:T * 256 + 256])
                    o_proj_pair(T)
                    return
                rb = psv.tile([65, 256], f32, tag="ps_pv")
                nc.tensor.matmul(
                    rb[0:64, :], ones64[64:65, :], rrow[64:65, :],
                    start=True, stop=True,
                )
                osb = rbpool.tile([128, 2 * D_MODEL], bf16, tag="osb")
                for half in range(2):
                    hs = slice(half * 128, (half + 1) * 128)
                    nc.vector.tensor_mul(
                        attnAB[0:64, T * 256 + half * 128:T * 256 + half * 128 + 128],
                        pvs[0:64, hs], rb[0:64, hs],
                    )
                    qtl = 2 * T + half
                    pot = pspool.tile([128, 1024], f32, tag="ps")
                    lhsAB = attnAB[:, qtl * 128:qtl * 128 + 128]
                    nc.tensor.matmul(
                        pot[:, 0:512], lhsAB, wo_sb[:, 0:512],
                        start=True, stop=True,
                    )
                    nc.tensor.matmul(
                        pot[:, 512:768], lhsAB, wo_sb[:, 512:768],
                        start=True, stop=True,
                    )
                    if half == 0:
                        nc.scalar.activation(
                            osb[:, 0:768], pot[:, 0:768],
                            Copy, bias=0.0, scale=1.0,
                        )
                    else:
                        nc.vector.tensor_copy(osb[:, 768:1536], pot[:, 0:768])
                    r0 = T * 256 + half * 128
                    last_opart[0] = nc.sync.dma_start(
                        out=o_part[r0:r0 + 128, :],
                        in_=osb[:, half * D_MODEL:(half + 1) * D_MODEL],
                    )

            def finalize_btile(m, handles):
                """Write the B attention into attnAB rows 64:128 at BOTH
                physical positions 2m and 2m+1; the parity one-hot in the
                broadcast ones-column (host data) zeroes the non-owned
                position so o_proj can use real head-B weights everywhere."""
                pvs, rrow = handles
                for po in range(2):
                    pos = 2 * m + po
                    rb = psv.tile([128, 256], f32, tag="ps_pv")
                    nc.tensor.matmul(
                        rb[64:128, :],
                        onesp_sb[32:33, po * 64:(po + 1) * 64],
                        rrow[32:33, :],
                        start=True, stop=True,
                    )
                    nc.vector.tensor_mul(
                        attnAB[64:128, pos * 256:(pos + 1) * 256],
                        pvs[64:128, :], rb[64:128, :],
                    )

            def o_proj_pair(T):
                osb = rbpool.tile([128, 2 * D_MODEL], bf16, tag="osb")
                for half in range(2):
                    qtl = 2 * T + half
                    pot = pspool.tile([128, 1024], f32, tag="ps")
                    # A (rows 0:64) and B (rows 64:128) contract together
                    lhsAB = attnAB[:, qtl * 128:qtl * 128 + 128]
                    nc.tensor.matmul(
                        pot[:, 0:512], lhsAB, wo_sb[:, 0:512],
                        start=True, stop=True,
                    )
                    nc.tensor.matmul(
                        pot[:, 512:768], lhsAB, wo_sb[:, 512:768],
                        start=True, stop=True,
                    )
                    # Act engine has slack during the small early tiles and
                    # DVE is the early bottleneck; late tiles are the
                    # reverse.  The very last tile splits across both
                    # engines so the final ReduceScatter fires sooner.
                    if (T < 10 or T >= 14) and not (
                        T == N_CH * 2 - 1 and half == 1
                    ):
                        nc.scalar.activation(
                            osb[:, half * D_MODEL:(half + 1) * D_MODEL],
                            pot[:, 0:768], Copy, bias=0.0, scale=1.0,
                        )
                    else:
                        nc.vector.tensor_copy(
                            osb[:, half * D_MODEL:(half + 1) * D_MODEL],
                            pot[:, 0:768],
                        )
                    # per-half DMA so the last half's write (and the final
                    # ReduceScatter behind it) starts as early as possible
                    r0 = T * 256 + half * 128
                    last_opart[0] = nc.sync.dma_start(
                        out=o_part[r0:r0 + 128, :],
                        in_=osb[:, half * D_MODEL:(half + 1) * D_MODEL],
                    )

            def fire_rs(j):
                _, lo, hi = RS_CHUNKS[j]
                nc.gpsimd.collective_compute(
                    "ReduceScatter",
                    mybir.AluOpType.add,
                    replica_groups=[list(range(N_CORES))],
                    ins=[o_part[lo:hi, :]],
                    outs=[ors[j][:]],
                )

            def readback(j):
                """Pure-DMA bounce ors -> SBUF -> out (both bf16)."""
                _, lo, hi = RS_CHUNKS[j]
                shard = (hi - lo) // 8
                oo = RS_OUT_OFF[j]
                rt = rbpool.tile([128, 4 * D_MODEL], bf16, tag="rt")
                rd = nc.sync.dma_start(
                    out=rt[0:shard // 4, :].rearrange(
                        "p (h d) -> p h d", d=D_MODEL
                    ),
                    in_=ors[j][:].rearrange("(p h) d -> p h d", h=4),
                )
                # keep readback DMAs after all o_part writes so the
                # round-robin DMA-queue counts of collective waits never
                # include collective-dependent transfers
                if last_opart[0] is not None:
                    add_dep_helper(
                        rd.ins, last_opart[0].ins, sync=True,
                        reason="readback after o_part stream",
                    )
                nc.sync.dma_start(
                    out=out_d[oo:oo + shard, :].rearrange(
                        "(p h) d -> p h d", h=4
                    ),
                    in_=rt[0:shard // 4, :].rearrange(
                        "p (h d) -> p h d", d=D_MODEL
                    ),
                )

            # ---- main loop: projection chunk ch, then attention q-tiles;
            # tile finalization (normalize + o_proj + RS) runs one tile behind
            rs_next = 0
            last_opart = [None]
            b_done = False
            pending = None  # (kind, idx, handles)

            def proj_pair(ch, w, ws, dst):
                # fp8 DoubleRow: each matmul contracts a PAIR of 128-row
                # feature blocks (weights/ifmap free dims are (2, m)/(2, s));
                # a tensor and its pair-swapped twin share one PSUM tile,
                # consumed by rope before the next pair allocates
                xt8_3d = xt8_sb[:].rearrange("p (e s) -> p e s", s=S)
                ck = cosk_sb[:, ch * 512:(ch + 1) * 512]
                sk = sink_sb[:, ch * 512:(ch + 1) * 512]
                ps = pspool.tile([128, 1024], f32, tag="ps")
                for off, wt in ((0, w), (512, ws)):
                    w3 = wt[:].rearrange("p (e m) -> p e m", m=128)
                    for j in range(EB // 2):
                        nc.tensor.matmul(
                            ps[:, off:off + 512],
                            w3[:, 2 * j:2 * j + 2, :],
                            xt8_3d[:, 2 * j:2 * j + 2, ch * 512:(ch + 1) * 512],
                            start=(j == 0),
                            stop=(j == EB // 2 - 1),
                            perf_mode=DR,
                        )
                rope(dst, ps[:, 0:512], ps[:, 512:1024], ck, sk)

            def do_proj_kq(ch):
                proj_pair(ch, wk_sb, wks_sb, ktc[ch][:])
                proj_pair(ch, wq_sb, wqs_sb, qt[:, ch * 512:(ch + 1) * 512])

            def do_proj_v(ch):
                def xt_t_slice(eb, lo, hi):
                    return xt_sb[:, eb * S + ch * 512 + lo:eb * S + ch * 512 + hi]
                # V (natural layout, interleaved ones column per head):
                # 4 s-tiles accumulate into one PSUM tile
                psV4 = pspool.tile([128, 1024], f32, tag="ps")
                for stl in range(4):
                    for eb in range(EB):
                        nc.tensor.matmul(
                            psV4[:, stl * 256:stl * 256 + 128],
                            xt_t_slice(eb, stl * 128, stl * 128 + 128),
                            wv_sb[:, eb * 128:(eb + 1) * 128],
                            start=(eb == 0),
                            stop=(eb == EB - 1),
                        )
                vall = vc[ch][:].rearrange("p (s x) -> p s x", s=4)
                nc.vector.memset(vall[:, :, 64:66], 1.0)   # Aone, Bone
                nc.vector.memset(vall[:, :, 66:97], 0.0)   # fill
                vsrc = psV4[:].rearrange("p (s x) -> p s x", s=4)[:, :, 0:128]
                # A data at cols 0:64, B data at cols 97:161
                nc.vector.tensor_copy(vall[:, :, 0:64], vsrc[:, :, 0:64])
                nc.vector.tensor_copy(vall[:, :, 97:161], vsrc[:, :, 64:128])

            def gather_qb(m):
                """qb tile m = roped head-B Q of physical q-tile 2m+parity,
                gathered from qt's 512-col window holding both parities'
                tiles (indices are per-core host data, so the instruction
                stream stays SPMD-uniform and m-independent)."""
                nc.gpsimd.ap_gather(
                    out_ap=qb[:, m * 256:(m + 1) * 256].rearrange(
                        "p (i d) -> p i d", d=16
                    ),
                    in_ap=qt[:, m * 512:(m + 1) * 512].rearrange(
                        "p (i d) -> p i d", d=16
                    ),
                    idxs_ap=idxb_sb[:, 0:1],
                    channels=128,
                    num_elems=32,
                    d=16,
                    num_idxs=16,
                )

            # projections run one chunk ahead, split and issued mid-chunk so
            # the attention tiles keep the Act engine fed at boundaries
            do_proj_kq(0)
            do_proj_v(0)
            gather_qb(0)
            do_proj_kq(1)
            do_proj_v(1)
            gather_qb(1)

            def pop_pending():
                nonlocal pending, rs_next, b_done
                if pending is None:
                    return
                kind, idx, ph = pending
                if kind == "A":
                    finalize_tile(idx, ph)
                    if (
                        with_rs
                        and rs_next < len(RS_CHUNKS)
                        and RS_CHUNKS[rs_next][0] == idx
                    ):
                        fire_rs(rs_next)
                        rs_next += 1
                else:
                    finalize_btile(idx, ph)
                pending = None

            warm = None
            for ch in range(N_CH):
                bh, warm = run_tile(spec_b(ch), warm, spec_a(2 * ch))
                if ch + 2 < N_CH:
                    do_proj_v(ch + 2)
                pop_pending()
                pending = ("B", ch, bh)
                for T in (2 * ch, 2 * ch + 1):
                    if T == 2 * ch:
                        nxt = spec_a(T + 1)
                    elif ch + 1 < N_CH:
                        nxt = spec_b(ch + 1)
                    else:
                        nxt = None
                    handles, warm = run_tile(spec_a(T), warm, nxt)
                    pop_pending()
                    pending = ("A", T, handles)
                    if T == 13:
                        # finalize now so the middle ReduceScatter fires
                        # before b7's issue stream and clears the exclusive
                        # collective device ahead of the final chunk
                        pop_pending()
                    if ch + 2 < N_CH:
                        if T == 2 * ch:
                            proj_pair(
                                ch + 2, wk_sb, wks_sb, ktc[ch + 2][:]
                            )
                        else:
                            proj_pair(
                                ch + 2, wq_sb, wqs_sb,
                                qt[:, (ch + 2) * 512:(ch + 3) * 512],
                            )
                            gather_qb(ch + 2)
            pop_pending()
            while with_rs and rs_next < len(RS_CHUNKS):
                fire_rs(rs_next)
                rs_next += 1
            if with_rs:
                for j in range(len(RS_CHUNKS)):
                    readback(j)


    nc.compile()
    return nc


_PROGRAM = None


def _get_program():
    global _PROGRAM
    if _PROGRAM is None:
        _PROGRAM = build_program()
    return _PROGRAM


def host_prep(in_features, token_positions, q_proj, k_proj, v_proj, o_proj):
    """Build the 8 per-core input maps."""
    x = np.asarray(in_features, np.float32).reshape(S, D_MODEL)
    tp = np.asarray(token_positions)
    qp = np.asarray(q_proj, np.float32)
    kp = np.asarray(k_proj, np.float32)
    vp = np.asarray(v_proj, np.float32)
    op = np.asarray(o_proj, np.float32)

    xt_bf = np.ascontiguousarray(x.T).astype(BF16)      # [768, 4096]
    xt_f8 = np.ascontiguousarray(x.T).astype(FP8)
    wqT = np.ascontiguousarray(qp.T)                    # [in 768, out 768]
    wkT = np.ascontiguousarray(kp.T)
    wvT = np.ascontiguousarray(vp.T)
    opT = np.ascontiguousarray(op.T)                    # [in-dk 768, out 768]

    inv_freq = 1.0 / THETA ** (np.arange(0, DK, 2, dtype=np.float32) / DK)
    pos = np.clip(tp.astype(np.float32), 0, MAX_SEQ_LEN - 1)
    freq = pos[:, None] * inv_freq[None, :]             # [S, 32]
    cos_t, sin_t = np.cos(freq), np.sin(freq)

    r = np.arange(128)
    fidx = (r % 64) // 2
    sign = np.where(r % 2 == 0, -1.0, 1.0).astype(np.float32)
    cos128 = cos_t[:, fidx].T.astype(BF16)              # [128, S]
    sin128 = (sin_t[:, fidx].T * sign[:, None]).astype(BF16)

    ki = np.arange(128)[:, None]
    qi = np.arange(256)[None, :]
    mask_a = (ki <= qi).astype(np.float32)
    mask_b = (ki + 128 <= qi).astype(np.float32)
    maskab = np.concatenate([mask_a, mask_b], axis=1).astype(BF16)

    ones512 = np.ones((128, 512), np.float32)
    zeros512 = np.zeros((128, 512), np.float32)

    in_maps = []
    for c in range(N_CORES):
        hA = c
        hB = 8 + c // 2
        p = c % 2

        def wslice(wT, dt=BF16):
            out = np.empty((D_MODEL, 128), np.float32)
            out[:, 0:64] = wT[:, hA * 64:(hA + 1) * 64]
            out[:, 64:128] = wT[:, hB * 64:(hB + 1) * 64]
            return out.astype(dt)

        wo2 = np.zeros((128, D_MODEL), np.float32)
        wo2[0:64, :] = opT[hA * 64:(hA + 1) * 64, :]
        wo2[64:128, :] = opT[hB * 64:(hB + 1) * 64, :]

        # parity one-hot ones-columns used to broadcast (and zero) the
        # head-B softmax reciprocals at positions 2m / 2m+1
        onesp = np.zeros((64, 128), np.float32)
        onesp[32, 0:64] = 1.0 if p == 0 else 0.0
        onesp[32, 64:128] = 1.0 if p == 1 else 0.0

        # ap_gather block indices: within each 512-col qt window, this
        # core's parity selects the first or second 256-col q-tile
        # (rows 0:64 gather head-A data into unused qb rows; keep valid)
        idxb = np.zeros((128, N_CH), np.int16)
        idxb[0:64, :] = (np.arange(64) % 16)[:, None]
        idxb[64:128, :] = (p * 16 + np.arange(64) % 16)[:, None]

        maskab_f = maskab.astype(np.float32)
        if p == 0:
            maskh = np.concatenate([maskab_f, zeros512], axis=1)
        else:
            maskh = np.concatenate([ones512, maskab_f], axis=1)

        in_maps.append(
            {
                "xt": xt_bf,
                "xt8": xt_f8,
                "idxb": idxb,
                "wq8": wslice(wqT, FP8),
                "wk8": wslice(wkT, FP8),
                "wq8s": wslice(wqT, FP8)[:, np.arange(128) ^ 1],
                "wk8s": wslice(wkT, FP8)[:, np.arange(128) ^ 1],
                "wv2": wslice(wvT),
                "wo2": wo2.astype(BF16),
                "onesp": onesp.astype(BF16),
                "cosk": cos128,
                "sink": sin128,
                "maskab": maskab,
                "maskh": maskh.astype(BF16),
            }
        )
    return in_maps


def assemble_output(results):
    out = np.empty((1, S, D_MODEL), np.float32)
    for c in range(N_CORES):
        r = np.asarray(results[c]["out"]).astype(np.float32)
        for j, (_, lo, hi) in enumerate(RS_CHUNKS):
            shard = (hi - lo) // 8
            oo = RS_OUT_OFF[j]
            out[0, lo + shard * c:lo + shard * (c + 1)] = r[oo:oo + shard]
    return out


def kernel(**inputs):
    from concourse.bass_utils import run_bass_kernel_spmd

    nc = _get_program()
    in_maps = host_prep(**inputs)
    res = run_bass_kernel_spmd(nc, in_maps, list(range(N_CORES)))
    return assemble_output(res.results)


if __name__ == "__main__":
    nc = build_program()
    print("program built and compiled")



# revision 97
# speedup vs baseline: 1.0011x; 1.0003x over previous
"""Trainium2 Bass kernel: causal MultiHeadAttention with RoPE (head-parallel).

B=1, S=4096, D=768, H=12 heads, dk=64, fp32 I/O. 8 NeuronCores, SPMD.

Sharding: head-parallel with split tail heads. Core c owns whole head
A=c (heads 0..7) plus HALF of head B=8+c//2: the q-tiles of parity c%2
(4 tiles of 256 rows each). Every core projects K/V/Q for its two heads
over the full sequence, runs full-causal attention for head A and its
four B q-tiles (identical instruction stream everywhere; the lone
parity-dependent bit is a [128,1024] mask input), computes bf16 partial
output projections, and combines them with ReduceScatter collectives:
an 8-way RS for the A-partials (q-sharded result) and a 4-way RS over
each parity group for the B-partials, which the host adds on top.
"""

import sys

if "/opt/trn_rl_repo" not in sys.path:
    sys.path.insert(0, "/opt/trn_rl_repo")

import numpy as np
import ml_dtypes

D_MODEL = 768
H = 12
DK = 64
S = 4096
THETA = 10000.0
MAX_SEQ_LEN = 4096
N_CORES = 8
EB = D_MODEL // 128   # 6 contraction blocks
N_CH = S // 512       # 8 sequence chunks
VW = 161              # V_aug s-tile layout: [Adata 64|Aone|Bone|fill 31|Bdata 64]
#  - A-PV lhsT window = cols 0:65   -> out rows 0:64 data, 64 denom
#  - B-PV lhsT window = cols 33:161 -> out row 32 denom, rows 64:128 data
#    (junk rows 0:32,33:64 unused; alignment puts B attn at partitions
#     64:128 so o_proj contracts A+B in ONE 128-deep matmul)

BF16 = ml_dtypes.bfloat16
FP8 = ml_dtypes.float8_e4m3   # matches mybir.dt.float8e4

# ReduceScatter chunks: fire after q-tile T_FIRE, covering q rows [lo, hi).
# Fired one q-tile after the covered rows complete so the collective's
# input-DMA waits are already satisfied and never block the Pool queue.
# Each RS writes its shard straight into the bf16 external output; chunk
# boundaries are chosen so each RS clears the (exclusive) collective
# device before the next one fires: cost = 15us flat + out_bytes/40GB/s.
RS_CHUNKS = [
    (11, 0, 2816),
    (13, 2816, 3584),
    (15, 3584, 4096),
]
# out_d row offset of each chunk's shard
RS_OUT_OFF = [0, 352, 448]


def build_program(with_rs=True):
    import concourse.mybir as mybir
    import concourse.tile as tile
    from concourse import bacc, library_config
    from concourse.tile import add_dep_helper

    f32 = mybir.dt.float32
    bf16 = mybir.dt.bfloat16
    fp8 = mybir.dt.float8e4
    DR = mybir.MatmulPerfMode.DoubleRow
    Exp = mybir.ActivationFunctionType.Exp
    Copy = mybir.ActivationFunctionType.Copy

    nc = bacc.Bacc(
        "TRN2",
        target_bir_lowering=False,
        debug=False,
        enable_asserts=True,
        num_devices=N_CORES,
    )

    xt_d = nc.dram_tensor("xt", [D_MODEL, S], bf16, kind="ExternalInput")
    xt8_d = nc.dram_tensor("xt8", [D_MODEL, S], fp8, kind="ExternalInput")
    w_d = {
        n: nc.dram_tensor(n, [D_MODEL, 128], bf16, kind="ExternalInput")
        for n in ("wv2",)
    }
    w8_d = {
        n: nc.dram_tensor(n, [D_MODEL, 128], fp8, kind="ExternalInput")
        for n in ("wq8", "wk8", "wq8s", "wk8s")
    }
    wo_d = nc.dram_tensor("wo2", [128, D_MODEL], bf16, kind="ExternalInput")
    onesp_d = nc.dram_tensor("onesp", [64, 128], bf16, kind="ExternalInput")
    idxb_d = nc.dram_tensor(
        "idxb", [128, N_CH], mybir.dt.int16, kind="ExternalInput"
    )
    maskh_d = nc.dram_tensor("maskh", [128, 1024], bf16, kind="ExternalInput")
    cosk_d = nc.dram_tensor("cosk", [128, S], bf16, kind="ExternalInput")
    sink_d = nc.dram_tensor("sink", [128, S], bf16, kind="ExternalInput")
    mask_d = nc.dram_tensor("maskab", [128, 512], bf16, kind="ExternalInput")
    out_d = nc.dram_tensor("out", [512, D_MODEL], bf16, kind="ExternalOutput")
    o_part = nc.dram_tensor("o_part", [S, D_MODEL], bf16, kind="Internal")
    ors = [
        nc.dram_tensor(f"ors{j}", [(hi - lo) // 8, D_MODEL], bf16, kind="Internal")
        for j, (_, lo, hi) in enumerate(RS_CHUNKS)
    ]


    PAIRSWAP = [i ^ 1 for i in range(32)]

    with tile.TileContext(nc) as tc:
        with (
            tc.tile_pool(name="const", bufs=1) as cpool,
            tc.tile_pool(name="rope", bufs=6) as rpool,
            tc.tile_pool(name="expp", bufs=8) as epool,
            tc.tile_pool(name="norm", bufs=8) as npool,
            tc.tile_pool(name="rsrb", bufs=2) as rbpool,
            tc.tile_pool(name="ps", bufs=3, space="PSUM") as pspool,
            tc.tile_pool(name="ps_pv", bufs=2, space="PSUM") as psv,
        ):
            nc.gpsimd.load_library(library_config.ap_gather)

            # ---- persistent tensors; first chunk's inputs loaded first ----
            def load_w(n):
                t = cpool.tile([128, EB * 128], bf16, tag=f"w_{n}", name=n)
                nc.sync.dma_start(
                    out=t[:].rearrange("p (e m) -> p e m", m=128),
                    in_=w_d[n][:].rearrange("(e p) m -> p e m", p=128),
                )
                return t

            def load_w8(n):
                t = cpool.tile([128, EB * 128], fp8, tag=f"w_{n}", name=n)
                nc.sync.dma_start(
                    out=t[:].rearrange("p (e m) -> p e m", m=128),
                    in_=w8_d[n][:].rearrange("(e p) m -> p e m", p=128),
                )
                return t

            def xt_load(lo, hi):
                nc.sync.dma_start(
                    out=xt_sb[:].rearrange("p (e s) -> p e s", s=S)[:, :, lo:hi],
                    in_=xt_d[:].rearrange("(e p) s -> p e s", p=128)[:, :, lo:hi],
                )

            def xt8_load(lo, hi):
                nc.sync.dma_start(
                    out=xt8_sb[:].rearrange("p (e s) -> p e s", s=S)[:, :, lo:hi],
                    in_=xt8_d[:].rearrange("(e p) s -> p e s", p=128)[:, :, lo:hi],
                )

            def cs_load(lo, hi):
                nc.sync.dma_start(out=cosk_sb[:, lo:hi], in_=cosk_d[:, lo:hi])
                nc.sync.dma_start(out=sink_sb[:, lo:hi], in_=sink_d[:, lo:hi])

            # chunk-0 K/Q inputs first, then V, then head-B / masks / wo,
            # then remaining chunks interleaved in first-use order
            wk_sb = load_w8("wk8")
            wks_sb = load_w8("wk8s")
            xt8_sb = cpool.tile([128, EB * S], fp8, tag="xt8_sb")
            xt8_load(0, 512)
            cosk_sb = cpool.tile([128, S], bf16, tag="cosk_sb")
            sink_sb = cpool.tile([128, S], bf16, tag="sink_sb")
            cs_load(0, 512)
            wq_sb = load_w8("wq8")
            wqs_sb = load_w8("wq8s")
            wv_sb = load_w("wv2")
            xt_sb = cpool.tile([128, EB * S], bf16, tag="xt_sb")
            xt_load(0, 512)
            maskab = cpool.tile([128, 512], bf16, tag="maskab")
            nc.sync.dma_start(out=maskab[:], in_=mask_d[:])
            idxb_sb = cpool.tile([128, N_CH], mybir.dt.int16, tag="idxb")
            nc.sync.dma_start(out=idxb_sb[:], in_=idxb_d[:])
            xt8_load(512, 1024)
            cs_load(512, 1024)
            maskh = cpool.tile([128, 1024], bf16, tag="maskh")
            nc.sync.dma_start(out=maskh[:], in_=maskh_d[:])
            xt_load(512, 1024)
            wo_sb = cpool.tile([128, D_MODEL], bf16, tag="wo2")
            nc.sync.dma_start(out=wo_sb[:], in_=wo_d[:])
            onesp_sb = cpool.tile([64, 128], bf16, tag="onesp")
            nc.sync.dma_start(out=onesp_sb[:], in_=onesp_d[:])
            for ch in range(2, N_CH):
                xt8_load(ch * 512, (ch + 1) * 512)
                cs_load(ch * 512, (ch + 1) * 512)
                xt_load(ch * 512, (ch + 1) * 512)

            qb = cpool.tile([128, 2048], bf16, tag="qb")
            ones64 = cpool.tile([65, 64], bf16, tag="ones64")
            nc.vector.memset(ones64[64:65, :], 1.0)
            qt = cpool.tile([128, S], bf16, tag="qt")
            ktc = [
                cpool.tile([128, 512], bf16, tag=f"kt{ch}", name=f"kt{ch}")
                for ch in range(N_CH)
            ]
            vc = [
                cpool.tile([128, 4 * VW], bf16, tag=f"va{ch}", name=f"va{ch}")
                for ch in range(N_CH)
            ]
            attnAB = cpool.tile([128, S], bf16, tag="attnAB", name="attnAB")

            def rope(dst, src_ps, swp_ps, cos_ap, sin_ap):
                # pair-swap comes from a second projection against
                # host-swapped weights; both muls read PSUM directly
                xb = rpool.tile([128, 512], bf16, tag="rope_x")
                nc.vector.tensor_mul(xb[:], src_ps, cos_ap)
                sh = rpool.tile([128, 512], bf16, tag="rope_sh")
                nc.vector.tensor_mul(sh[:], swp_ps, sin_ap)
                nc.vector.tensor_add(dst, xb[:], sh[:])

            # ---- warm-chained attention tiles: each tile pre-issues the
            # NEXT tile's first score-group + exp before its own final PV
            # batch, so the Act engine never drains at tile boundaries.
            def spec_a(T):
                return ("A", T,
                        [(pg, 2 if pg + 1 <= T else 1)
                         for pg in range(0, T + 1, 2)])

            def spec_b(m):
                return ("B", m, [(pg, 2) for pg in range(0, 2 * m + 2, 2)])

            def issue_group(spec, pg, w):
                """Score matmuls + exp (+ causal mask) for one group."""
                kind, idx, _ = spec
                ro = 0 if kind == "A" else 64
                if kind == "A":
                    qsl = qt[0:64, idx * 256:idx * 256 + 256]
                else:
                    qsl = qb[64:128, idx * 256:idx * 256 + 256]
                sc = pspool.tile([128, 1024], f32, tag="ps")
                for pi in range(w):
                    for j in range(2):
                        t = 2 * (pg + pi) + j
                        nc.tensor.matmul(
                            sc[:, (2 * pi + j) * 256:(2 * pi + j + 1) * 256],
                            ktc[t // 4][ro:ro + 64, (t % 4) * 128:(t % 4) * 128 + 128],
                            qsl,
                            start=True,
                            stop=True,
                        )
                et = epool.tile([128, 1024], bf16, tag="et")
                nc.scalar.activation(
                    et[:, 0:512 * w], sc[:, 0:512 * w], Exp, bias=0.0, scale=0.125
                )
                if kind == "A":
                    if pg + w - 1 == idx:  # group holds the diagonal pair
                        off = 512 * (w - 1)
                        nc.vector.tensor_mul(
                            et[:, off:off + 512], et[:, off:off + 512], maskab[:]
                        )
                else:
                    if pg + 2 >= 2 * idx + 2:  # diagonal + padding pair
                        nc.vector.tensor_mul(et[:], et[:], maskh[:])
                return et

            def issue_pv(spec, pv, pg, w, et):
                kind, idx, _ = spec
                last_p = idx if kind == "A" else 2 * idx + 1
                for pi in range(w):
                    p = pg + pi
                    for j in range(2):
                        t = 2 * p + j
                        if kind == "A":
                            lhsT = vc[t // 4][:, (t % 4) * VW:(t % 4) * VW + 65]
                        else:
                            lhsT = vc[t // 4][:, (t % 4) * VW + 33:(t % 4) * VW + 161]
                        nc.tensor.matmul(
                            pv[:],
                            lhsT,
                            et[:, (2 * pi + j) * 256:(2 * pi + j + 1) * 256],
                            start=(p == 0 and j == 0),
                            stop=(p == last_p and j == 1),
                        )

            def run_tile(spec, warm, next_spec):
                kind, idx, groups = spec
                pv = psv.tile(
                    [65 if kind == "A" else 128, 256], f32, tag="ps_pv"
                )
                # up to 2 groups were pre-issued by the previous tile;
                # PV issue lags the exp issue by the same depth
                pend = []
                if warm:
                    for (pg, w), et in zip(groups, warm):
                        pend.append((pg, w, et))
                for pg, w in groups[len(pend):]:
                    et = issue_group(spec, pg, w)
                    if pend:
                        issue_pv(spec, pv, *pend.pop(0))
                    pend.append((pg, w, et))
                warm_next = None
                if next_spec is not None:
                    warm_next = [
                        issue_group(next_spec, pg, w)
                        for pg, w in next_spec[2][:3]
                    ]
                for h in pend:
                    issue_pv(spec, pv, *h)
                # reduce + reciprocal now; broadcast and normalize deferred
                if kind == "A":
                    pvs = npool.tile([65, 256], f32, tag="pvs")
                    nc.vector.tensor_copy(pvs[:], pv[:])
                    rrow = npool.tile([65, 256], bf16, tag="rrow")
                    with nc.allow_low_precision(reason="bf16 softmax denom"):
                        nc.vector.reciprocal(rrow[64:65, :], pvs[64:65, :])
                else:
                    pvs = npool.tile([128, 256], f32, tag="pvs")
                    nc.vector.tensor_copy(pvs[64:128, :], pv[64:128, :])
                    rrow = npool.tile([65, 256], bf16, tag="rrow")
                    with nc.allow_low_precision(reason="bf16 softmax denom"):
                        # denominator read straight from PSUM row 32
                        nc.vector.reciprocal(rrow[32:33, :], pv[32:33, :])
                return (pvs, rrow), warm_next

            def normalize(pvs, rrow, dst):
                rb = psv.tile([65, 256], f32, tag="ps_pv")
                nc.tensor.matmul(
                    rb[0:64, :], ones64[64:65, :], rrow[64:65, :],
                    start=True, stop=True,
                )
                nc.vector.tensor_mul(dst, pvs[0:64, :], rb[0:64, :])

            def finalize_tile(T, handles):
                """Deferred normalize (broadcast via K=1 matmul) + output
                projection for q-tile T; issued one tile later so the
                reciprocal is ready and the PE never waits.  The last tile
                pipelines normalize/o_proj/copy/DMA per 128-row half so the
                final ReduceScatter fires as early as possible."""
                pvs, rrow = handles
                if T < N_CH * 2 - 1:
                    normalize(pvs, rrow, attnAB[0:64, T * 256:T * 256 + 256])
                    o_proj_pair(T)
                    return
                rb = psv.tile([65, 256], f32, tag="ps_pv")
                nc.tensor.matmul(
                    rb[0:64, :], ones64[64:65, :], rrow[64:65, :],
                    start=True, stop=True,
                )
                osb = rbpool.tile([128, 2 * D_MODEL], bf16, tag="osb")
                for half in range(2):
                    hs = slice(half * 128, (half + 1) * 128)
                    nc.vector.tensor_mul(
                        attnAB[0:64, T * 256 + half * 128:T * 256 + half * 128 + 128],
                        pvs[0:64, hs], rb[0:64, hs],
                    )
                    qtl = 2 * T + half
                    pot = pspool.tile([128, 1024], f32, tag="ps")
                    lhsAB = attnAB[:, qtl * 128:qtl * 128 + 128]
                    nc.tensor.matmul(
                        pot[:, 0:512], lhsAB, wo_sb[:, 0:512],
                        start=True, stop=True,
                    )
                    nc.tensor.matmul(
                        pot[:, 512:768], lhsAB, wo_sb[:, 512:768],
                        start=True, stop=True,
                    )
                    if half == 0:
                        nc.scalar.activation(
                            osb[:, 0:768], pot[:, 0:768],
                            Copy, bias=0.0, scale=1.0,
                        )
                    else:
                        nc.vector.tensor_copy(osb[:, 768:1536], pot[:, 0:768])
                    r0 = T * 256 + half * 128
                    last_opart[0] = nc.sync.dma_start(
                        out=o_part[r0:r0 + 128, :],
                        in_=osb[:, half * D_MODEL:(half + 1) * D_MODEL],
                    )

            def finalize_btile(m, handles):
                """Write the B attention into attnAB rows 64:128 at BOTH
                physical positions 2m and 2m+1; the parity one-hot in the
                broadcast ones-column (host data) zeroes the non-owned
                position so o_proj can use real head-B weights everywhere."""
                pvs, rrow = handles
                for po in range(2):
                    pos = 2 * m + po
                    rb = psv.tile([128, 256], f32, tag="ps_pv")
                    nc.tensor.matmul(
                        rb[64:128, :],
                        onesp_sb[32:33, po * 64:(po + 1) * 64],
                        rrow[32:33, :],
                        start=True, stop=True,
                    )
                    nc.vector.tensor_mul(
                        attnAB[64:128, pos * 256:(pos + 1) * 256],
                        pvs[64:128, :], rb[64:128, :],
                    )

            def o_proj_pair(T):
                osb = rbpool.tile([128, 2 * D_MODEL], bf16, tag="osb")
                for half in range(2):
                    qtl = 2 * T + half
                    pot = pspool.tile([128, 1024], f32, tag="ps")
                    # A (rows 0:64) and B (rows 64:128) contract together
                    lhsAB = attnAB[:, qtl * 128:qtl * 128 + 128]
                    nc.tensor.matmul(
                        pot[:, 0:512], lhsAB, wo_sb[:, 0:512],
                        start=True, stop=True,
                    )
                    nc.tensor.matmul(
                        pot[:, 512:768], lhsAB, wo_sb[:, 512:768],
                        start=True, stop=True,
                    )
                    # Act engine has slack during the small early tiles and
                    # DVE is the early bottleneck; late tiles are the
                    # reverse.  The very last tile splits across both
                    # engines so the final ReduceScatter fires sooner.
                    if (T < 10 or T >= 14) and not (
                        T == N_CH * 2 - 1 and half == 1
                    ):
                        nc.scalar.activation(
                            osb[:, half * D_MODEL:(half + 1) * D_MODEL],
                            pot[:, 0:768], Copy, bias=0.0, scale=1.0,
                        )
                    else:
                        nc.vector.tensor_copy(
                            osb[:, half * D_MODEL:(half + 1) * D_MODEL],
                            pot[:, 0:768],
                        )
                    # per-half DMA so the last half's write (and the final
                    # ReduceScatter behind it) starts as early as possible
                    r0 = T * 256 + half * 128
                    last_opart[0] = nc.sync.dma_start(
                        out=o_part[r0:r0 + 128, :],
                        in_=osb[:, half * D_MODEL:(half + 1) * D_MODEL],
                    )

            def fire_rs(j):
                _, lo, hi = RS_CHUNKS[j]
                nc.gpsimd.collective_compute(
                    "ReduceScatter",
                    mybir.AluOpType.add,
                    replica_groups=[list(range(N_CORES))],
                    ins=[o_part[lo:hi, :]],
                    outs=[ors[j][:]],
                )

            def readback(j):
                """Pure-DMA bounce ors -> SBUF -> out (both bf16)."""
                _, lo, hi = RS_CHUNKS[j]
                shard = (hi - lo) // 8
                oo = RS_OUT_OFF[j]
                rt = rbpool.tile([128, 4 * D_MODEL], bf16, tag="rt")
                rd = nc.sync.dma_start(
                    out=rt[0:shard // 4, :].rearrange(
                        "p (h d) -> p h d", d=D_MODEL
                    ),
                    in_=ors[j][:].rearrange("(p h) d -> p h d", h=4),
                )
                # keep readback DMAs after all o_part writes so the
                # round-robin DMA-queue counts of collective waits never
                # include collective-dependent transfers
                if last_opart[0] is not None:
                    add_dep_helper(
                        rd.ins, last_opart[0].ins, sync=True,
                        reason="readback after o_part stream",
                    )
                nc.sync.dma_start(
                    out=out_d[oo:oo + shard, :].rearrange(
                        "(p h) d -> p h d", h=4
                    ),
                    in_=rt[0:shard // 4, :].rearrange(
                        "p (h d) -> p h d", d=D_MODEL
                    ),
                )

            # ---- main loop: projection chunk ch, then attention q-tiles;
            # tile finalization (normalize + o_proj + RS) runs one tile behind
            rs_next = 0
            last_opart = [None]
            b_done = False
            pending = None  # (kind, idx, handles)

            def proj_pair(ch, w, ws, dst):
                # fp8 DoubleRow: each matmul contracts a PAIR of 128-row
                # feature blocks (weights/ifmap free dims are (2, m)/(2, s));
                # a tensor and its pair-swapped twin share one PSUM tile,
                # consumed by rope before the next pair allocates
                xt8_3d = xt8_sb[:].rearrange("p (e s) -> p e s", s=S)
                ck = cosk_sb[:, ch * 512:(ch + 1) * 512]
                sk = sink_sb[:, ch * 512:(ch + 1) * 512]
                ps = pspool.tile([128, 1024], f32, tag="ps")
                for off, wt in ((0, w), (512, ws)):
                    w3 = wt[:].rearrange("p (e m) -> p e m", m=128)
                    for j in range(EB // 2):
                        nc.tensor.matmul(
                            ps[:, off:off + 512],
                            w3[:, 2 * j:2 * j + 2, :],
                            xt8_3d[:, 2 * j:2 * j + 2, ch * 512:(ch + 1) * 512],
                            start=(j == 0),
                            stop=(j == EB // 2 - 1),
                            perf_mode=DR,
                        )
                rope(dst, ps[:, 0:512], ps[:, 512:1024], ck, sk)

            def do_proj_kq(ch):
                proj_pair(ch, wk_sb, wks_sb, ktc[ch][:])
                proj_pair(ch, wq_sb, wqs_sb, qt[:, ch * 512:(ch + 1) * 512])

            def do_proj_v(ch):
                def xt_t_slice(eb, lo, hi):
                    return xt_sb[:, eb * S + ch * 512 + lo:eb * S + ch * 512 + hi]
                # V (natural layout, interleaved ones column per head):
                # 4 s-tiles accumulate into one PSUM tile
                psV4 = pspool.tile([128, 1024], f32, tag="ps")
                for stl in range(4):
                    for eb in range(EB):
                        nc.tensor.matmul(
                            psV4[:, stl * 256:stl * 256 + 128],
                            xt_t_slice(eb, stl * 128, stl * 128 + 128),
                            wv_sb[:, eb * 128:(eb + 1) * 128],
                            start=(eb == 0),
                            stop=(eb == EB - 1),
                        )
                vall = vc[ch][:].rearrange("p (s x) -> p s x", s=4)
                nc.vector.memset(vall[:, :, 64:66], 1.0)   # Aone, Bone
                nc.vector.memset(vall[:, :, 66:97], 0.0)   # fill
                vsrc = psV4[:].rearrange("p (s x) -> p s x", s=4)[:, :, 0:128]
                # A data at cols 0:64, B data at cols 97:161
                nc.vector.tensor_copy(vall[:, :, 0:64], vsrc[:, :, 0:64])
                nc.vector.tensor_copy(vall[:, :, 97:161], vsrc[:, :, 64:128])

            def gather_qb(m):
                """qb tile m = roped head-B Q of physical q-tile 2m+parity,
                gathered from qt's 512-col window holding both parities'
                tiles (indices are per-core host data, so the instruction
                stream stays SPMD-uniform and m-independent)."""
                nc.gpsimd.ap_gather(
                    out_ap=qb[:, m * 256:(m + 1) * 256].rearrange(
                        "p (i d) -> p i d", d=16
                    ),
                    in_ap=qt[:, m * 512:(m + 1) * 512].rearrange(
                        "p (i d) -> p i d", d=16
                    ),
                    idxs_ap=idxb_sb[:, 0:1],
                    channels=128,
                    num_elems=32,
                    d=16,
                    num_idxs=16,
                )

            # projections run one chunk ahead, split and issued mid-chunk so
            # the attention tiles keep the Act engine fed at boundaries
            do_proj_kq(0)
            do_proj_v(0)
            gather_qb(0)
            do_proj_kq(1)
            do_proj_v(1)
            gather_qb(1)

            def pop_pending():
                nonlocal pending, rs_next, b_done
                if pending is None:
                    return
                kind, idx, ph = pending
                if kind == "A":
                    finalize_tile(idx, ph)
                    if (
                        with_rs
                        and rs_next < len(RS_CHUNKS)
                        and RS_CHUNKS[rs_next][0] == idx
                    ):
                        fire_rs(rs_next)
                        rs_next += 1
                else:
                    finalize_btile(idx, ph)
                pending = None

            warm = None
            for ch in range(N_CH):
                bh, warm = run_tile(spec_b(ch), warm, spec_a(2 * ch))
                if ch + 2 < N_CH:
                    do_proj_v(ch + 2)
                pop_pending()
                pending = ("B", ch, bh)
                for T in (2 * ch, 2 * ch + 1):
                    if T == 2 * ch:
                        nxt = spec_a(T + 1)
                    elif ch + 1 < N_CH:
                        nxt = spec_b(ch + 1)
                    else:
                        nxt = None
                    handles, warm = run_tile(spec_a(T), warm, nxt)
                    pop_pending()
                    pending = ("A", T, handles)
                    if T == 13:
                        # finalize now so the middle ReduceScatter fires
                        # before b7's issue stream and clears the exclusive
                        # collective device ahead of the final chunk
                        pop_pending()
                    if ch + 2 < N_CH:
                        if T == 2 * ch:
                            proj_pair(
                                ch + 2, wk_sb, wks_sb, ktc[ch + 2][:]
                            )
                        else:
                            proj_pair(
                                ch + 2, wq_sb, wqs_sb,
                                qt[:, (ch + 2) * 512:(ch + 3) * 512],
                            )
                            gather_qb(ch + 2)
            pop_pending()
            while with_rs and rs_next < len(RS_CHUNKS):
                fire_rs(rs_next)
                rs_next += 1
            if with_rs:
                for j in range(len(RS_CHUNKS)):
                    readback(j)


    nc.compile()
    return nc


_PROGRAM = None


def _get_program():
    global _PROGRAM
    if _PROGRAM is None:
        _PROGRAM = build_program()
    return _PROGRAM


def host_prep(in_features, token_positions, q_proj, k_proj, v_proj, o_proj):
    """Build the 8 per-core input maps."""
    x = np.asarray(in_features, np.float32).reshape(S, D_MODEL)
    tp = np.asarray(token_positions)
    qp = np.asarray(q_proj, np.float32)
    kp = np.asarray(k_proj, np.float32)
    vp = np.asarray(v_proj, np.float32)
    op = np.asarray(o_proj, np.float32)

    xt_bf = np.ascontiguousarray(x.T).astype(BF16)      # [768, 4096]
    xt_f8 = np.ascontiguousarray(x.T).astype(FP8)
    wqT = np.ascontiguousarray(qp.T)                    # [in 768, out 768]
    wkT = np.ascontiguousarray(kp.T)
    wvT = np.ascontiguousarray(vp.T)
    opT = np.ascontiguousarray(op.T)                    # [in-dk 768, out 768]

    inv_freq = 1.0 / THETA ** (np.arange(0, DK, 2, dtype=np.float32) / DK)
    pos = np.clip(tp.astype(np.float32), 0, MAX_SEQ_LEN - 1)
    freq = pos[:, None] * inv_freq[None, :]             # [S, 32]
    cos_t, sin_t = np.cos(freq), np.sin(freq)

    r = np.arange(128)
    fidx = (r % 64) // 2
    sign = np.where(r % 2 == 0, -1.0, 1.0).astype(np.float32)
    cos128 = cos_t[:, fidx].T.astype(BF16)              # [128, S]
    sin128 = (sin_t[:, fidx].T * sign[:, None]).astype(BF16)

    ki = np.arange(128)[:, None]
    qi = np.arange(256)[None, :]
    mask_a = (ki <= qi).astype(np.float32)
    mask_b = (ki + 128 <= qi).astype(np.float32)
    maskab = np.concatenate([mask_a, mask_b], axis=1).astype(BF16)

    ones512 = np.ones((128, 512), np.float32)
    zeros512 = np.zeros((128, 512), np.float32)

    in_maps = []
    for c in range(N_CORES):
        hA = c
        hB = 8 + c // 2
        p = c % 2

        def wslice(wT, dt=BF16):
            out = np.empty((D_MODEL, 128), np.float32)
            out[:, 0:64] = wT[:, hA * 64:(hA + 1) * 64]
            out[:, 64:128] = wT[:, hB * 64:(hB + 1) * 64]
            return out.astype(dt)

        wo2 = np.zeros((128, D_MODEL), np.float32)
        wo2[0:64, :] = opT[hA * 64:(hA + 1) * 64, :]
        wo2[64:128, :] = opT[hB * 64:(hB + 1) * 64, :]

        # parity one-hot ones-columns used to broadcast (and zero) the
        # head-B softmax reciprocals at positions 2m / 2m+1
        onesp = np.zeros((64, 128), np.float32)
        onesp[32, 0:64] = 1.0 if p == 0 else 0.0
        onesp[32, 64:128] = 1.0 if p == 1 else 0.0

        # ap_gather block indices: within each 512-col qt window, this
        # core's parity selects the first or second 256-col q-tile
        # (rows 0:64 gather head-A data into unused qb rows; keep valid)
        idxb = np.zeros((128, N_CH), np.int16)
        idxb[0:64, :] = (np.arange(64) % 16)[:, None]
        idxb[64:128, :] = (p * 16 + np.arange(64) % 16)[:, None]

        maskab_f = maskab.astype(np.float32)
        if p == 0:
            maskh = np.concatenate([maskab_f, zeros512], axis=1)
        else:
            maskh = np.concatenate([ones512, maskab_f], axis=1)

        in_maps.append(
            {
                "xt": xt_bf,
                "xt8": xt_f8,
                "idxb": idxb,
                "wq8": wslice(wqT, FP8),
                "wk8": wslice(wkT, FP8),
                "wq8s": wslice(wqT, FP8)[:, np.arange(128) ^ 1],
                "wk8s": wslice(wkT, FP8)[:, np.arange(128) ^ 1],
                "wv2": wslice(wvT),
                "wo2": wo2.astype(BF16),
                "onesp": onesp.astype(BF16),
                "cosk": cos128,
                "sink": sin128,
                "maskab": maskab,
                "maskh": maskh.astype(BF16),
            }
        )
    return in_maps


def assemble_output(results):
    out = np.empty((1, S, D_MODEL), np.float32)
    for c in range(N_CORES):
        r = np.asarray(results[c]["out"]).astype(np.float32)
        for j, (_, lo, hi) in enumerate(RS_CHUNKS):
            shard = (hi - lo) // 8
            oo = RS_OUT_OFF[j]
            out[0, lo + shard * c:lo + shard * (c + 1)] = r[oo:oo + shard]
    return out


def kernel(**inputs):
    from concourse.bass_utils import run_bass_kernel_spmd

    nc = _get_program()
    in_maps = host_prep(**inputs)
    res = run_bass_kernel_spmd(nc, in_maps, list(range(N_CORES)))
    return assemble_output(res.results)


if __name__ == "__main__":
    nc = build_program()
    print("program built and compiled")

